# revision 6
# baseline (speedup 1.0000x reference)
"""Multi-head attention (B=4, S=2048, D=1024, H=16, dh=64) on 8 trn2 NeuronCores.

Sharding: core i handles batch b=i//2 and head-group g=i%2 (8 heads each).
Each core computes, for its (b, g):
  qT/kT: transposed per-head projections stored head-pair-packed [128, S]
  v:     natural-layout projections [S, 512] as 16 tiles [128, 512] (bf16)
  scores^T = kT.T-contraction (row-tiled 64x128 PE, fp32)
  p = exp(scores/8) (ACT, bf16, un-normalized softmax: inputs are small so
      max-subtraction is unnecessary)
  outT = v.T @ p accumulated in PSUM (col-tiled 128x64 PE), normalized by
      rowsums (gpsimd partition_all_reduce over the accumulated p) on evac
  out_part = outT.T @ Wo_g  (final projection, partial over head-group)
Host sums the two group partials per batch and adds bo.
"""

import os
import numpy as np

B, S, D, H = 4, 2048, 1024, 16
DH = 64          # head dim
G = 2            # head groups (tensor-parallel dimension)
HG = H // G      # 8 heads per group
P = 128          # partitions
SB = 512         # matmul free-dim block
NPAIR = HG // 2  # 4 head pairs per core
ND = D // P      # 8 contraction tiles for projections
NT = S // P      # 16 key/value tiles
NU = 2           # units per pair: 1024 s-columns each
UW = S // NU     # 1024

LAST_EXEC_NS = None
LAST_RESULTS = None
_NC_CACHE = {}


def _build_nc():
    import concourse.bacc as bacc
    import concourse.mybir as mybir
    import concourse.tile as tile

    f32 = mybir.dt.float32
    bf16 = mybir.dt.bfloat16
    Exp = mybir.ActivationFunctionType.Exp
    from concourse import bass_isa

    nc = bacc.Bacc("TRN2")

    qt_d = nc.dram_tensor("qt", [D, S], f32, kind="ExternalInput")
    kt_d = nc.dram_tensor("kt", [D, S], f32, kind="ExternalInput")
    vt_d = nc.dram_tensor("vt", [D, S], f32, kind="ExternalInput")
    wq_d = nc.dram_tensor("wq", [D, HG * DH], f32, kind="ExternalInput")
    wk_d = nc.dram_tensor("wk", [D, HG * DH], f32, kind="ExternalInput")
    wv_d = nc.dram_tensor("wv", [D, HG * DH], f32, kind="ExternalInput")
    wo_d = nc.dram_tensor("wo", [HG * DH, D], f32, kind="ExternalInput")
    bq_d = nc.dram_tensor("bq", [HG * DH, 1], f32, kind="ExternalInput")
    bk_d = nc.dram_tensor("bk", [HG * DH, 1], f32, kind="ExternalInput")
    bv_d = nc.dram_tensor("bv", [HG * DH, 1], f32, kind="ExternalInput")
    id_d = nc.dram_tensor("ident", [P, P], bf16, kind="ExternalInput")
    out_d = nc.dram_tensor("out_part", [S, D], f32, kind="ExternalOutput")

    with tile.TileContext(nc) as tc:
        with (
            tc.tile_pool(name="persist", bufs=1) as pers,
            tc.tile_pool(name="stream", bufs=1) as strm,
            tc.tile_pool(name="psum", bufs=1, space="PSUM") as psum,
        ):
            # ---------------- persistent tiles ----------------
            qT = [pers.tile([P, S], bf16, name=f"qT{p}") for p in range(NPAIR)]
            kT = [pers.tile([P, S], bf16, name=f"kT{p}") for p in range(NPAIR)]
            vv = [pers.tile([P, HG * DH], bf16, name=f"vv{t}") for t in range(NT)]
            oT = [pers.tile([P, S], f32, name=f"oT{p}") for p in range(NPAIR)]
            wo_s = [pers.tile([P, D], f32, name=f"wo{p}") for p in range(NPAIR)]
            ident = pers.tile([P, P], bf16, name="ident")

            nc.sync.dma_start(ident[:], id_d[:])
            for p in range(NPAIR):
                nc.sync.dma_start(wo_s[p][:], wo_d[p * P:(p + 1) * P, :])

            # ---------------- phase 1: projections ----------------
            # order: k, v, q — so the main loop (which needs full kT and vv,
            # but only the first column-block of qT) can start earliest.
            for xd, wd, bd, kind in ((kt_d, wk_d, bk_d, "k"),
                                     (vt_d, wv_d, bv_d, "v"),
                                     (qt_d, wq_d, bq_d, "q")):
                wts = []
                for d in range(ND):
                    w = strm.tile([P, HG * DH], f32, tag="w", bufs=10,
                                  name=f"w_{kind}{d}")
                    nc.sync.dma_start(w[:], wd[d * P:(d + 1) * P, :])
                    wts.append(w)
                bias = []
                for p in range(NPAIR):
                    b = strm.tile([P, 1], f32, tag="bias", bufs=6,
                                  name=f"b_{kind}{p}")
                    nc.sync.dma_start(b[:], bd[p * P:(p + 1) * P, :])
                    bias.append(b)
                vtmp = [None] * NPAIR
                if kind == "v":
                    for p in range(NPAIR):
                        vtmp[p] = strm.tile([P, S], bf16, tag="vtmp", bufs=4,
                                            name=f"vtmp{p}")
                for sb in range(S // SB):
                    xts = []
                    for d in range(ND):
                        x = strm.tile([P, SB], f32, tag="x", bufs=12,
                                      name=f"x_{kind}{sb}_{d}")
                        nc.sync.dma_start(
                            x[:], xd[d * P:(d + 1) * P, sb * SB:(sb + 1) * SB])
                        xts.append(x)
                    for p in range(NPAIR):
                        ps = psum.tile([P, 2 * SB], f32, tag="sc", bufs=3,
                                       name=f"ps_{kind}{sb}_{p}")
                        for d in range(ND):
                            nc.tensor.matmul(
                                ps[:, 0:SB],
                                wts[d][:, p * P:(p + 1) * P],
                                xts[d][:],
                                start=(d == 0), stop=(d == ND - 1))
                        if kind == "q":
                            dest = qT[p]
                        elif kind == "k":
                            dest = kT[p]
                        else:
                            dest = vtmp[p]
                        nc.vector.tensor_scalar_add(
                            dest[:, sb * SB:(sb + 1) * SB], ps[:, 0:SB],
                            bias[p])
                if kind == "v":
                    # transpose vtmp [e,s] -> natural v [s,e] tiles
                    for p in range(NPAIR):
                        for st in range(NT):
                            tp = psum.tile([P, P], bf16, tag="av", bufs=2,
                                           name=f"tp{p}_{st}")
                            nc.tensor.transpose(
                                tp[:], vtmp[p][:, st * P:(st + 1) * P],
                                ident[:])
                            nc.vector.tensor_copy(
                                vv[st][:, p * P:(p + 1) * P], tp[:])

            # ---------------- main loop: attention ----------------
            for u in range(NU):
                for p in range(NPAIR):
                    col0 = u * UW
                    av_lo = psum.tile([P, SB], f32, tag="av", bufs=2,
                                      name=f"avlo{u}_{p}")
                    av_hi = psum.tile([P, SB], f32, tag="av", bufs=2,
                                      name=f"avhi{u}_{p}")
                    acc_e = strm.tile([P, UW], bf16, tag="acc", bufs=4,
                                      name=f"acce{u}_{p}")
                    acc_o = strm.tile([P, UW], bf16, tag="acc", bufs=4,
                                      name=f"acco{u}_{p}")
                    for t in range(NT):
                        pts = []
                        for half, base in ((0, 0), (1, 64)):
                            sc = psum.tile([P, UW], f32, tag="sc", bufs=3,
                                           name=f"sc{u}_{p}_{t}_{half}")
                            lhsT = kT[p][base:base + 64, t * P:(t + 1) * P]
                            nc.tensor.matmul(
                                sc[:, 0:SB],
                                lhsT, qT[p][base:base + 64, col0:col0 + SB],
                                start=True, stop=True)
                            nc.tensor.matmul(
                                sc[:, SB:UW],
                                lhsT, qT[p][base:base + 64,
                                            col0 + SB:col0 + UW],
                                start=True, stop=True)
                            pt = strm.tile([P, UW], bf16, tag="pt", bufs=6,
                                           name=f"pt{u}_{p}_{t}_{half}")
                            nc.scalar.activation(pt[:], sc[:], Exp,
                                                 scale=0.125)
                            pts.append(pt)
                        pt_e, pt_o = pts
                        if t == 0:
                            nc.vector.tensor_copy(acc_e[:], pt_e[:])
                            nc.vector.tensor_copy(acc_o[:], pt_o[:])
                        else:
                            nc.vector.tensor_add(acc_e[:], acc_e[:], pt_e[:])
                            nc.vector.tensor_add(acc_o[:], acc_o[:], pt_o[:])
                        ve = vv[t][:, p * P:p * P + DH]
                        vo = vv[t][:, p * P + DH:(p + 1) * P]
                        st_, sp_ = (t == 0), (t == NT - 1)
                        nc.tensor.matmul(av_lo[0:64, :], ve, pt_e[:, 0:SB],
                                         start=st_, stop=sp_)
                        nc.tensor.matmul(av_hi[0:64, :], ve, pt_e[:, SB:UW],
                                         start=st_, stop=sp_)
                        nc.tensor.matmul(av_lo[64:P, :], vo, pt_o[:, 0:SB],
                                         start=st_, stop=sp_)
                        nc.tensor.matmul(av_hi[64:P, :], vo, pt_o[:, SB:UW],
                                         start=st_, stop=sp_)
                    # rowsums -> reciprocal -> normalize into oT
                    rs_e = strm.tile([P, UW], f32, tag="rs", bufs=2,
                                     name=f"rse{u}_{p}")
                    rs_o = strm.tile([P, UW], f32, tag="rs", bufs=2,
                                     name=f"rso{u}_{p}")
                    nc.gpsimd.partition_all_reduce(
                        rs_e[:], acc_e[:], channels=P,
                        reduce_op=bass_isa.ReduceOp.add)
                    nc.gpsimd.partition_all_reduce(
                        rs_o[:], acc_o[:], channels=P,
                        reduce_op=bass_isa.ReduceOp.add)
                    rb = strm.tile([P, UW], f32, tag="rb", bufs=2,
                                   name=f"rb{u}_{p}")
                    nc.vector.reciprocal(rb[0:64, :], rs_e[0:64, :])
                    nc.vector.reciprocal(rb[64:P, :], rs_o[64:P, :])
                    nc.vector.tensor_mul(oT[p][0:64, col0:col0 + SB],
                                         av_lo[0:64, :], rb[0:64, 0:SB])
                    nc.vector.tensor_mul(oT[p][0:64, col0 + SB:col0 + UW],
                                         av_hi[0:64, :], rb[0:64, SB:UW])
                    nc.vector.tensor_mul(oT[p][64:P, col0:col0 + SB],
                                         av_lo[64:P, :], rb[64:P, 0:SB])
                    nc.vector.tensor_mul(oT[p][64:P, col0 + SB:col0 + UW],
                                         av_hi[64:P, :], rb[64:P, SB:UW])
                # ---------------- final projection for this unit ----------
                for st in range(u * (NT // NU), (u + 1) * (NT // NU)):
                    for nb in range(D // SB):
                        fin = psum.tile([P, 2 * SB], f32, tag="sc", bufs=3,
                                        name=f"fin{st}_{nb}")
                        for p in range(NPAIR):
                            nc.tensor.matmul(
                                fin[:, 0:SB],
                                oT[p][:, st * P:(st + 1) * P],
                                wo_s[p][:, nb * SB:(nb + 1) * SB],
                                start=(p == 0), stop=(p == NPAIR - 1))
                        fo = strm.tile([P, SB], f32, tag="fo", bufs=3,
                                       name=f"fo{st}_{nb}")
                        nc.vector.tensor_copy(fo[:], fin[:, 0:SB])
                        nc.sync.dma_start(
                            out_d[st * P:(st + 1) * P, nb * SB:(nb + 1) * SB],
                            fo[:])
    nc.compile()
    return nc


def _host_prep(inputs):
    import ml_dtypes
    Q = np.asarray(inputs["Query"], dtype=np.float32)
    K = np.asarray(inputs["Key"], dtype=np.float32)
    V = np.asarray(inputs["Value"], dtype=np.float32)
    Wq = np.asarray(inputs["Wq"], dtype=np.float32)
    Wk = np.asarray(inputs["Wk"], dtype=np.float32)
    Wv = np.asarray(inputs["Wv"], dtype=np.float32)
    bq = np.asarray(inputs["bq"], dtype=np.float32)
    bk = np.asarray(inputs["bk"], dtype=np.float32)
    bv = np.asarray(inputs["bv"], dtype=np.float32)
    Wo = np.asarray(inputs["Wo"], dtype=np.float32)

    ident = np.eye(P, dtype=ml_dtypes.bfloat16)
    in_maps = []
    for core in range(8):
        b, g = core // G, core % G
        hs = slice(g * HG, (g + 1) * HG)
        # head-major column packing [D, HG*DH]
        wq_g = np.ascontiguousarray(
            Wq[hs].transpose(1, 0, 2).reshape(D, HG * DH))
        wk_g = np.ascontiguousarray(
            Wk[hs].transpose(1, 0, 2).reshape(D, HG * DH))
        wv_g = np.ascontiguousarray(
            Wv[hs].transpose(1, 0, 2).reshape(D, HG * DH))
        in_maps.append({
            "qt": np.ascontiguousarray(Q[b].T),
            "kt": np.ascontiguousarray(K[b].T),
            "vt": np.ascontiguousarray(V[b].T),
            "wq": wq_g, "wk": wk_g, "wv": wv_g,
            "wo": np.ascontiguousarray(Wo[g * HG * DH:(g + 1) * HG * DH, :]),
            "bq": np.ascontiguousarray(bq[hs].reshape(HG * DH, 1)),
            "bk": np.ascontiguousarray(bk[hs].reshape(HG * DH, 1)),
            "bv": np.ascontiguousarray(bv[hs].reshape(HG * DH, 1)),
            "ident": ident,
        })
    return in_maps


def kernel(**inputs):
    global LAST_EXEC_NS, LAST_RESULTS
    from concourse.bass_utils import run_bass_kernel_spmd

    if "nc" not in _NC_CACHE:
        _NC_CACHE["nc"] = _build_nc()
    nc = _NC_CACHE["nc"]

    in_maps = _host_prep(inputs)
    trace = bool(int(os.environ.get("BASS_KERNEL_TRACE", "0")))
    res = run_bass_kernel_spmd(nc, in_maps, list(range(8)), trace=trace)
    LAST_EXEC_NS = res.exec_time_ns
    LAST_RESULTS = res

    bo = np.asarray(inputs["bo"], dtype=np.float32)
    out = np.empty((B, S, D), dtype=np.float32)
    for b in range(B):
        out[b] = (res.results[G * b]["out_part"]
                  + res.results[G * b + 1]["out_part"] + bo)
    return out
